# revision 34
# baseline (speedup 1.0000x reference)
"""Multi-head attention (B=2, N=2048, C=1024, H=16) on 8 Trainium2 NeuronCores.

Sharding: tensor-parallel over heads (2 heads/core) for qkv-proj + attention;
all-to-all of the attention output (split into 4 half-batch collectives,
pipelined under attention), then each core runs the output projection over
the full channel dim for its token slices.  Host concatenates slices.

Device layouts (per core, heads A=2c, B=2c+1):
  x^T  [c_in, tok]          via HW DMA-transpose (bf16), spread across the
                            sync/scalar/vector/gpsimd queues so descriptor
                            generation parallelizes (all hoisted before the
                            first collective - Tile serializes DMA-transposes
                            against collectives)
  q^T/k^T [128, tok]        rows 0-63 head A, 64-127 head B (from w as lhsT)
  v    [tok, 130]           cols: [v_A|1|v_B|1]  (ones col -> softmax denom)
  S^T  [j, i] = k^T.T @ q^T per head; the two heads run CONCURRENTLY as
                            K=64 row-tiled matmuls (tile_position (0,0) and
                            (64,0)) -> ~2x the S-phase throughput
  expS = exp(0.125 * S^T)   (ScalarE, bf16)
  out_u^T [65, i] = [v|1].T @ expS  (rows 0-63 out, row 64 = denominator)
  normalize: both heads' denominators packed in a [2,512] tile -> one
  reciprocal_approx_fast + one K=2 selector-matmul broadcast (E2) per i-tile

The emission order software-pipelines the engine queues (which execute in
order): normalize/o_proj work is deferred behind later attention tiles so the
TensorE queue never waits on the DVE reciprocal chain or on a collective.
"""

import numpy as np
import ml_dtypes
from contextlib import ExitStack

import concourse.bass as bass
import concourse.tile as tile
from concourse import bacc, mybir
from concourse.bass_utils import run_bass_kernel_spmd
from concourse.masks import make_identity

BF16 = mybir.dt.bfloat16
F32 = mybir.dt.float32
EXP = mybir.ActivationFunctionType.Exp
NPBF16 = ml_dtypes.bfloat16

NCORES = 8
B, NSEQ, C, H, D = 2, 2048, 1024, 16, 64
T = B * NSEQ                 # 4096 flattened tokens
SCALE = D ** -0.5            # folded into the exp activation
NKC = C // 128               # 8 contraction chunks
ITILE = 512                  # query tile (free dim of S^T)
NI = NSEQ // ITILE           # 4 i-tiles per batch
NJ = NSEQ // 128             # 16 key chunks per batch
JG = 1                       # j-chunks per exp group (2-bank S tiles, so the
                             # combined 2-head S tile can double-buffer)
HALF = 1024                  # tokens per all-to-all (half batch)
TFRAG = HALF // NCORES       # 128 tokens per core per all-to-all
TSL = B * NSEQ // NCORES     # 512 output tokens per core
DEBUG = False                # adds a dram dump of outT (pre-a2a attention)


def build_program():
    nc = bacc.Bacc("TRN2", target_bir_lowering=False, debug=False,
                   num_devices=NCORES)

    x_d = nc.dram_tensor("x", [T, C], BF16, kind="ExternalInput")
    wqk_d = nc.dram_tensor("wqk", [C, 256], BF16, kind="ExternalInput")
    wv_d = nc.dram_tensor("wv", [C, 128], BF16, kind="ExternalInput")
    wp_d = nc.dram_tensor("wproj", [C, C], BF16, kind="ExternalInput")
    bp_d = nc.dram_tensor("bproj", [1, C], BF16, kind="ExternalInput")
    y_d = nc.dram_tensor("y", [TSL, C], F32, kind="ExternalOutput")

    # four half-batch all-to-all units (128 tok/core each)
    CC_TF = [TFRAG] * 4
    CC_HS = [0, HALF, NSEQ, NSEQ + HALF]
    CC_YR = [0, TFRAG, TSL // B, TSL // B + TFRAG]
    a2a_in = [nc.dram_tensor(f"a2a_in{q}", [NCORES * 128, CC_TF[q]], BF16)
              for q in range(4)]
    a2a_out = [nc.dram_tensor(f"a2a_out{q}", [NCORES * 128, CC_TF[q]], BF16)
               for q in range(4)]
    dbg_d = (nc.dram_tensor("dbg", [128, T], BF16, kind="ExternalOutput")
             if DEBUG else None)

    with tile.TileContext(nc) as tc, ExitStack() as ctx:
        ep = ctx.enter_context          # shorthand

        consts = ep(tc.tile_pool(name="consts", bufs=1))
        p_xt = ep(tc.tile_pool(name="xt", bufs=4))
        p_qk = ep(tc.tile_pool(name="qkt", bufs=2))
        p_vt = ep(tc.tile_pool(name="vt", bufs=2))
        p_v = ep(tc.tile_pool(name="vnat", bufs=2 * NJ))
        p_exp = ep(tc.tile_pool(name="exps", bufs=3))
        p_outt = ep(tc.tile_pool(name="outt", bufs=1))
        p_ouc = ep(tc.tile_pool(name="ouc", bufs=6))
        p_small = ep(tc.tile_pool(name="small", bufs=2))
        p_ots = ep(tc.tile_pool(name="ots", bufs=2))
        p_y = ep(tc.tile_pool(name="ysb", bufs=2))
        ps_mm = ep(tc.tile_pool(name="psmm", bufs=2, space="PSUM"))
        ps_s = ep(tc.tile_pool(name="pss", bufs=2, space="PSUM"))
        ps_ou = ep(tc.tile_pool(name="psou", bufs=2, space="PSUM"))

        # ---- qkv weights first (small, gate the first matmul) ----
        wqk_sb = consts.tile([128, NKC * 256], BF16)
        wv_sb = consts.tile([128, NKC * 128], BF16)
        wp_sb = consts.tile([128, NKC * C], BF16)
        bp_sb = consts.tile([1, C], BF16)
        for c in range(NKC):
            r = slice(c * 128, (c + 1) * 128)
            nc.sync.dma_start(out=wqk_sb[:, c * 256:(c + 1) * 256],
                              in_=wqk_d[r, :])
            nc.sync.dma_start(out=wv_sb[:, c * 128:(c + 1) * 128],
                              in_=wv_d[r, :])

        ident = consts.tile([128, 128], BF16)
        make_identity(nc, ident[:])
        ones_row = consts.tile([1, 128], BF16)
        nc.vector.memset(ones_row[:], 1.0)

        # ---- ALL x^T transposes up front on the sync queue only.  DMA
        # completion semaphores are SHARED across engine queues with
        # issue-order thresholds, so splitting transposes across queues
        # makes consumers fire early on the other queue's increments. ----
        xtp = []
        for b in range(B):
            for tp in range(2):
                xti = p_xt.tile([128, NKC * 1024], BF16, tag="xt", name="xti")
                for c in range(NKC):
                    nc.sync.dma_start_transpose(
                        xti[:, c * 1024:(c + 1) * 1024],
                        x_d[b * NSEQ + tp * 1024: b * NSEQ + (tp + 1) * 1024,
                            c * 128:(c + 1) * 128])
                xtp.append(xti)

        # w_proj + bias after the transposes (not needed until oproj(0))
        for c in range(NKC):
            nc.sync.dma_start(out=wp_sb[:, c * C:(c + 1) * C],
                              in_=wp_d[c * 128:(c + 1) * 128, :])
        nc.sync.dma_start(out=bp_sb[:], in_=bp_d[0:1, :])

        # persistent / cross-stage state
        outT = p_outt.tile([128, T], BF16, tag="outT", name="outT")
        qkT = {}      # b -> (q2, k2)  [128, NSEQ] packed both heads
        vTs = {}      # b -> vT
        vns = {}      # b -> [vn tiles]
        oucs = {}     # (b, i, h) -> flushed numerator SBUF tile [64, ITILE]
        dts = {}      # (b, i) -> [2, ITILE] denominators (row h)
        rcpbs = {}    # (b, i) -> [2, ITILE] bf16 reciprocals

        def xts(b, tt, c):
            xti = xtp[b * 2 + tt // 2]
            off = c * 1024 + (tt % 2) * ITILE
            return xti[:, off: off + ITILE]

        def qkv_tp(b, tp):
            if b not in qkT:
                q2 = p_qk.tile([128, NSEQ], BF16, tag="q2", name="q2")
                k2 = p_qk.tile([128, NSEQ], BF16, tag="k2", name="k2")
                vT = p_vt.tile([128, NSEQ], BF16, tag="vT", name="vT")
                qkT[b] = (q2, k2)
                vTs[b] = vT
            q2, k2 = qkT[b]
            vT = vTs[b]
            for w in range(2):          # 0 -> q, 1 -> k
                dst = q2 if w == 0 else k2
                pst = [ps_mm.tile([128, ITILE], F32, tag="mm", name="pst")
                       for _ in range(2)]
                for c in range(NKC):
                    for u in range(2):
                        nc.tensor.matmul(
                            pst[u][:],
                            wqk_sb[:, c * 256 + w * 128: c * 256 + (w + 1) * 128],
                            xts(b, 2 * tp + u, c),
                            start=(c == 0), stop=(c == NKC - 1))
                for u in range(2):
                    tsl2 = slice((2 * tp + u) * ITILE, (2 * tp + u + 1) * ITILE)
                    nc.vector.tensor_copy(dst[:, tsl2], pst[u][:])
            pst = [ps_mm.tile([128, ITILE], F32, tag="mm", name="pst")
                   for _ in range(2)]
            for c in range(NKC):
                for u in range(2):
                    nc.tensor.matmul(
                        pst[u][:],
                        wv_sb[:, c * 128:(c + 1) * 128],
                        xts(b, 2 * tp + u, c),
                        start=(c == 0), stop=(c == NKC - 1))
            for u in range(2):
                nc.vector.tensor_copy(
                    vT[:, (2 * tp + u) * ITILE:(2 * tp + u + 1) * ITILE],
                    pst[u][:])

        def vn_block(b, tcns):
            vT = vTs[b]
            vn = vns.setdefault(b, [None] * NJ)
            for tcn in tcns:
                vtr = ps_mm.tile([128, 2 * ITILE], BF16, tag="mm", name="vtr")
                nc.tensor.transpose(vtr[:, 0:128],
                                    vT[:, tcn * 128:(tcn + 1) * 128], ident[:])
                vni = p_v.tile([128, 193], BF16, tag="v", name="vni")
                nc.vector.memset(vni[:, 64:65], 1.0)
                nc.vector.memset(vni[:, 129:130], 1.0)
                nc.vector.memset(vni[:, 130:193], 0.0)
                nc.vector.tensor_copy(vni[:, 0:64], vtr[:, 0:64])
                nc.vector.tensor_copy(vni[:, 65:129], vtr[:, 64:128])
                vn[tcn] = vni

        attn_outu = {}

        def attn_part(b, i, groups):
            q2, k2 = qkT[b]
            vn = vns[b]
            isl = slice(i * ITILE, (i + 1) * ITILE)
            if (b, i) not in attn_outu:
                attn_outu[(b, i)] = [ps_ou.tile([128, ITILE], F32, tag="outu",
                                                name="outu")
                                     for _ in range(2)]
            outu = attn_outu[(b, i)]
            for g in groups:
                # ONE psum tile for BOTH heads' S -> the pair of row-tiled
                # matmuls shares its WAR dependency and stays adjacent in the
                # schedule, so the K=64 tile_position concurrency engages
                s2 = ps_s.tile([128, 2 * JG * ITILE], F32, tag="s",
                               name="sps")
                # 2-head row-tiled S: two concurrent K=64 matmuls on row
                # groups (0,0) and (64,0) -> full PE activity, no zero-pad
                for k in range(JG):
                    j = g * JG + k
                    jsl = slice(j * 128, (j + 1) * 128)
                    nc.tensor.matmul(
                        s2[:, k * ITILE:(k + 1) * ITILE],
                        k2[0:64, jsl], q2[0:64, isl],
                        start=True, stop=True, tile_position=(0, 0))
                    nc.tensor.matmul(
                        s2[:, JG * ITILE + k * ITILE:
                           JG * ITILE + (k + 1) * ITILE],
                        k2[64:128, jsl], q2[64:128, isl],
                        start=True, stop=True, tile_position=(64, 0))
                for h in range(2):
                    ex = p_exp.tile([128, JG * ITILE], BF16, tag=f"ex{h}",
                                    name="ex")
                    nc.scalar.activation(
                        ex[:], s2[:, h * JG * ITILE:(h + 1) * JG * ITILE],
                        EXP, scale=SCALE)
                    for k in range(JG):
                        j = g * JG + k
                        nc.tensor.matmul(
                            outu[h][:],
                            vn[j][:, h * 65: h * 65 + 128],
                            ex[:, k * ITILE:(k + 1) * ITILE],
                            start=(j == 0), stop=(j == NJ - 1))

        def attn_flush(b, i):
            """Flush outu psum -> SBUF; batch both heads' denominators and
            start the reciprocal chain immediately (off the PE queue)."""
            outu = attn_outu.pop((b, i))
            # both heads' denominators packed along the free dim on
            # partition 0 (reciprocal_approx_fast misbehaves off partition 0)
            dt = p_small.tile([1, 2 * ITILE], F32, tag="dt", name="dt")
            for h in range(2):
                ouc = p_ouc.tile([64, ITILE], F32, tag="ouc", name="ouc")
                nc.vector.tensor_copy(ouc[:], outu[h][0:64, :])
                nc.vector.tensor_copy(dt[0:1, h * ITILE:(h + 1) * ITILE],
                                      outu[h][64:65, :])
                oucs[(b, i, h)] = ouc
            rcp = p_small.tile([1, 2 * ITILE], F32, tag="rcp", name="rcp")
            nc.vector.reciprocal_approx_fast(out=rcp[:], in_=dt[:])
            rcpb = p_small.tile([1, 2 * ITILE], BF16, tag="rcpb", name="rcpb")
            nc.vector.tensor_copy(rcpb[:], rcp[:])
            dts[(b, i)] = dt
            rcpbs[(b, i)] = rcpb

        def attn(b, i):
            attn_part(b, i, range(NJ // JG))
            attn_flush(b, i)

        def normB(b, i):
            """Broadcast 1/denom to 64 partitions per head (two K=1 matmuls
            into one psum tile) and scale the numerators into outT."""
            t0 = b * NSEQ
            rcpb = rcpbs.pop((b, i))
            bc2 = ps_mm.tile([128, ITILE], F32, tag="mm", name="bc2")
            nc.tensor.matmul(bc2[0:64, :], ones_row[0:1, 0:64],
                             rcpb[0:1, 0:ITILE], start=True, stop=True)
            nc.tensor.matmul(bc2[64:128, :], ones_row[0:1, 64:128],
                             rcpb[0:1, ITILE:2 * ITILE],
                             start=True, stop=True)
            for h in range(2):
                ouc = oucs.pop((b, i, h))
                nc.vector.tensor_mul(
                    outT[h * 64:(h + 1) * 64,
                         t0 + i * ITILE: t0 + (i + 1) * ITILE],
                    ouc[:], bc2[h * 64:(h + 1) * 64, :])

        def stage_a2a(q):
            hs = CC_HS[q]
            nc.sync.dma_start(
                out=a2a_in[q][:, :].rearrange("(s p) t -> p s t", s=NCORES),
                in_=outT[:, hs: hs + NCORES * CC_TF[q]].rearrange(
                    "p (s t) -> p s t", s=NCORES))
            nc.gpsimd.collective_compute(
                "AllToAll", mybir.AluOpType.bypass,
                replica_groups=[list(range(NCORES))],
                ins=[a2a_in[q][:, :]], outs=[a2a_out[q][:, :]])

        def oproj(q):
            tf = CC_TF[q]
            ots = p_ots.tile([128, NCORES * TFRAG], BF16, tag="ots", name="ots")
            nc.sync.dma_start(
                out=ots[:, 0:NCORES * tf].rearrange("p (s t) -> p s t",
                                                    s=NCORES),
                in_=a2a_out[q][:, :].rearrange("(s p) t -> p s t", s=NCORES))
            y_ps = [ps_mm.tile([128, ITILE], F32, tag="mm", name="yps")
                    for _ in range(2)]
            for s in range(NKC):
                for n in range(2):
                    nc.tensor.matmul(
                        y_ps[n][0:tf, :],
                        ots[:, s * tf:(s + 1) * tf],
                        wp_sb[:, s * C + n * ITILE: s * C + (n + 1) * ITILE],
                        start=(s == 0), stop=False)
            for n in range(2):
                nc.tensor.matmul(y_ps[n][0:tf, :], ones_row[0:1, 0:tf],
                                 bp_sb[:, n * ITILE:(n + 1) * ITILE],
                                 start=False, stop=True)
            y_sb = p_y.tile([128, C], F32, tag="y", name="ysb")
            for n in range(2):
                nc.vector.tensor_copy(y_sb[0:tf, n * ITILE:(n + 1) * ITILE],
                                      y_ps[n][0:tf, :])
            yr0 = CC_YR[q]
            nc.sync.dma_start(out=y_d[yr0: yr0 + tf, :], in_=y_sb[0:tf, :])

        # ---- software-pipelined emission schedule ----
        G = NJ // JG
        qkv_tp(0, 0)
        vn_block(0, range(NJ // 2))
        attn_part(0, 0, range(G // 2))
        qkv_tp(0, 1)
        vn_block(0, range(NJ // 2, NJ))
        attn_part(0, 0, range(G // 2, G)); attn_flush(0, 0)
        attn(0, 1)
        normB(0, 0); normB(0, 1); stage_a2a(0)
        attn_part(0, 2, range(G // 2))
        qkv_tp(1, 0)
        attn_part(0, 2, range(G // 2, G)); attn_flush(0, 2)
        attn_part(0, 3, range(G // 2)); oproj(0)
        attn_part(0, 3, range(G // 2, G)); attn_flush(0, 3); normB(0, 2)
        qkv_tp(1, 1)
        vn_block(1, range(NJ))
        normB(0, 3); stage_a2a(1)
        attn(1, 0); oproj(1)
        attn(1, 1)
        normB(1, 0); normB(1, 1); stage_a2a(2)
        attn_part(1, 2, range(G // 2))
        attn_part(1, 2, range(G // 2, G)); attn_flush(1, 2)
        attn_part(1, 3, range(G // 2)); oproj(2)
        attn_part(1, 3, range(G // 2, G)); attn_flush(1, 3); normB(1, 2)
        normB(1, 3); stage_a2a(3)
        oproj(3)
        if DEBUG:
            nc.sync.dma_start(out=dbg_d[:, :], in_=outT[:, :])

    nc.compile()
    return nc


_NC = None


def _get_nc():
    global _NC
    if _NC is None:
        _NC = build_program()
    return _NC


def prep_in_maps(x, w_qkv, w_proj, b_proj):
    x_bf = np.ascontiguousarray(np.asarray(x, dtype=np.float32).reshape(T, C)
                                ).astype(NPBF16)
    w_qkv = np.asarray(w_qkv, dtype=np.float32)
    w_proj = np.asarray(w_proj, dtype=np.float32)
    b_proj = np.asarray(b_proj, dtype=np.float32)
    wp_bf = np.ascontiguousarray(w_proj).astype(NPBF16)
    bp_bf = b_proj.reshape(1, C).astype(NPBF16)

    q_w, k_w, v_w = w_qkv[:, 0:C], w_qkv[:, C:2 * C], w_qkv[:, 2 * C:3 * C]
    in_maps = []
    for c in range(NCORES):
        hA, hB = 2 * c, 2 * c + 1
        sA, sB = slice(hA * D, (hA + 1) * D), slice(hB * D, (hB + 1) * D)
        wqk_c = np.concatenate([q_w[:, sA], q_w[:, sB], k_w[:, sA], k_w[:, sB]],
                               axis=1).astype(NPBF16)
        wv_c = np.concatenate([v_w[:, sA], v_w[:, sB]], axis=1).astype(NPBF16)
        in_maps.append({"x": x_bf, "wqk": np.ascontiguousarray(wqk_c),
                        "wv": np.ascontiguousarray(wv_c), "wproj": wp_bf,
                        "bproj": bp_bf})
    return in_maps


CC_TF_H = [TFRAG] * 4
CC_HS_H = [0, HALF, NSEQ, NSEQ + HALF]
CC_YR_H = [0, TFRAG, TSL // B, TSL // B + TFRAG]


def assemble(results):
    y = np.empty((T, C), dtype=np.float32)
    for c in range(NCORES):
        yc = results[c]["y"]
        for q in range(4):
            tf = CC_TF_H[q]
            g0 = CC_HS_H[q] + c * tf
            r0 = CC_YR_H[q]
            y[g0: g0 + tf, :] = yc[r0: r0 + tf, :]
    return y.reshape(B, NSEQ, C)


def run(in_maps, trace=False):
    nc = _get_nc()
    return run_bass_kernel_spmd(nc, in_maps, core_ids=list(range(NCORES)),
                                trace=trace)


def kernel(x, w_qkv, w_proj, b_proj):
    res = run(prep_in_maps(x, w_qkv, w_proj, b_proj))
    return assemble(res.results)


# revision 36
# speedup vs baseline: 1.0320x; 1.0320x over previous
"""Multi-head attention (B=2, N=2048, C=1024, H=16) on 8 Trainium2 NeuronCores.

Sharding: tensor-parallel over heads (2 heads/core) for qkv-proj + attention;
all-to-all of the attention output (split into 4 half-batch collectives,
pipelined under attention), then each core runs the output projection over
the full channel dim for its token slices.  Host concatenates slices.

Device layouts (per core, heads A=2c, B=2c+1):
  x^T  [c_in, tok]          via HW DMA-transpose (bf16), spread across the
                            sync/scalar/vector/gpsimd queues so descriptor
                            generation parallelizes (all hoisted before the
                            first collective - Tile serializes DMA-transposes
                            against collectives)
  q^T/k^T [128, tok]        rows 0-63 head A, 64-127 head B (from w as lhsT)
  v    [tok, 130]           cols: [v_A|1|v_B|1]  (ones col -> softmax denom)
  S^T  [j, i] = k^T.T @ q^T per head; the two heads run CONCURRENTLY as
                            K=64 row-tiled matmuls (tile_position (0,0) and
                            (64,0)) -> ~2x the S-phase throughput
  expS = exp(0.125 * S^T)   (ScalarE, bf16)
  out_u^T [65, i] = [v|1].T @ expS  (rows 0-63 out, row 64 = denominator)
  normalize: both heads' denominators packed in a [2,512] tile -> one
  reciprocal_approx_fast + one K=2 selector-matmul broadcast (E2) per i-tile

The emission order software-pipelines the engine queues (which execute in
order): normalize/o_proj work is deferred behind later attention tiles so the
TensorE queue never waits on the DVE reciprocal chain or on a collective.
"""

import numpy as np
import ml_dtypes
from contextlib import ExitStack

import concourse.bass as bass
import concourse.tile as tile
from concourse import bacc, mybir
from concourse.bass_utils import run_bass_kernel_spmd
from concourse.masks import make_identity

BF16 = mybir.dt.bfloat16
F32 = mybir.dt.float32
EXP = mybir.ActivationFunctionType.Exp
NPBF16 = ml_dtypes.bfloat16

NCORES = 8
B, NSEQ, C, H, D = 2, 2048, 1024, 16, 64
T = B * NSEQ                 # 4096 flattened tokens
SCALE = D ** -0.5            # folded into the exp activation
NKC = C // 128               # 8 contraction chunks
ITILE = 512                  # query tile (free dim of S^T)
NI = NSEQ // ITILE           # 4 i-tiles per batch
NJ = NSEQ // 128             # 16 key chunks per batch
JG = 1                       # j-chunks per exp group (2-bank S tiles, so the
                             # combined 2-head S tile can double-buffer)
HALF = 1024                  # tokens per all-to-all (half batch)
TFRAG = HALF // NCORES       # 128 tokens per core per all-to-all
TSL = B * NSEQ // NCORES     # 512 output tokens per core
DEBUG = False                # adds a dram dump of outT (pre-a2a attention)


def build_program():
    nc = bacc.Bacc("TRN2", target_bir_lowering=False, debug=False,
                   num_devices=NCORES)

    x_d = nc.dram_tensor("x", [T, C], BF16, kind="ExternalInput")
    wqk_d = nc.dram_tensor("wqk", [C, 256], BF16, kind="ExternalInput")
    wv_d = nc.dram_tensor("wv", [C, 128], BF16, kind="ExternalInput")
    wp_d = nc.dram_tensor("wproj", [C, C], BF16, kind="ExternalInput")
    bp_d = nc.dram_tensor("bproj", [1, C], BF16, kind="ExternalInput")
    y_d = nc.dram_tensor("y", [TSL, C], F32, kind="ExternalOutput")

    # four half-batch all-to-all units (128 tok/core each)
    CC_TF = [TFRAG] * 4
    CC_HS = [0, HALF, NSEQ, NSEQ + HALF]
    CC_YR = [0, TFRAG, TSL // B, TSL // B + TFRAG]
    a2a_in = [nc.dram_tensor(f"a2a_in{q}", [NCORES * 128, CC_TF[q]], BF16)
              for q in range(4)]
    a2a_out = [nc.dram_tensor(f"a2a_out{q}", [NCORES * 128, CC_TF[q]], BF16)
               for q in range(4)]
    dbg_d = (nc.dram_tensor("dbg", [128, T], BF16, kind="ExternalOutput")
             if DEBUG else None)

    with tile.TileContext(nc) as tc, ExitStack() as ctx:
        ep = ctx.enter_context          # shorthand

        consts = ep(tc.tile_pool(name="consts", bufs=1))
        p_xt = ep(tc.tile_pool(name="xt", bufs=4))
        p_qk = ep(tc.tile_pool(name="qkt", bufs=2))
        p_vt = ep(tc.tile_pool(name="vt", bufs=2))
        p_v = ep(tc.tile_pool(name="vnat", bufs=2 * NJ))
        p_exp = ep(tc.tile_pool(name="exps", bufs=3))
        p_outt = ep(tc.tile_pool(name="outt", bufs=1))
        p_ouc = ep(tc.tile_pool(name="ouc", bufs=6))
        p_small = ep(tc.tile_pool(name="small", bufs=2))
        p_ots = ep(tc.tile_pool(name="ots", bufs=2))
        p_y = ep(tc.tile_pool(name="ysb", bufs=2))
        ps_mm = ep(tc.tile_pool(name="psmm", bufs=2, space="PSUM"))
        ps_s = ep(tc.tile_pool(name="pss", bufs=2, space="PSUM"))
        ps_ou = ep(tc.tile_pool(name="psou", bufs=2, space="PSUM"))

        # ---- qkv weights first (small, gate the first matmul) ----
        wqk_sb = consts.tile([128, NKC * 256], BF16)
        wv_sb = consts.tile([128, NKC * 128], BF16)
        wp_sb = consts.tile([128, NKC * C], BF16)
        bp_sb = consts.tile([1, C], BF16)
        for c in range(NKC):
            r = slice(c * 128, (c + 1) * 128)
            nc.sync.dma_start(out=wqk_sb[:, c * 256:(c + 1) * 256],
                              in_=wqk_d[r, :])
            nc.sync.dma_start(out=wv_sb[:, c * 128:(c + 1) * 128],
                              in_=wv_d[r, :])

        ident = consts.tile([128, 128], BF16)
        make_identity(nc, ident[:])
        ones_row = consts.tile([1, 128], BF16)
        nc.vector.memset(ones_row[:], 1.0)

        # ---- ALL x^T transposes up front on the sync queue only.  DMA
        # completion semaphores are SHARED across engine queues with
        # issue-order thresholds, so splitting transposes across queues
        # makes consumers fire early on the other queue's increments. ----
        xtp = []
        for b in range(B):
            for tp in range(2):
                xti = p_xt.tile([128, NKC * 1024], BF16, tag="xt", name="xti")
                # the first tile gates the whole pipeline: use [512,128]
                # transpose units so the first qkv matmuls start sooner
                nh = 2 if (b == 0 and tp == 0) else 1
                for c in range(NKC):
                    for hf in range(nh):
                        r0 = b * NSEQ + tp * 1024 + hf * (1024 // nh)
                        nc.sync.dma_start_transpose(
                            xti[:, c * 1024 + hf * (1024 // nh):
                                c * 1024 + (hf + 1) * (1024 // nh)],
                            x_d[r0: r0 + 1024 // nh,
                                c * 128:(c + 1) * 128])
                xtp.append(xti)

        # w_proj + bias after the transposes (not needed until oproj(0))
        for c in range(NKC):
            nc.sync.dma_start(out=wp_sb[:, c * C:(c + 1) * C],
                              in_=wp_d[c * 128:(c + 1) * 128, :])
        nc.sync.dma_start(out=bp_sb[:], in_=bp_d[0:1, :])

        # persistent / cross-stage state
        outT = p_outt.tile([128, T], BF16, tag="outT", name="outT")
        qkT = {}      # b -> (q2, k2)  [128, NSEQ] packed both heads
        vTs = {}      # b -> vT
        vns = {}      # b -> [vn tiles]
        oucs = {}     # (b, i, h) -> flushed numerator SBUF tile [64, ITILE]
        dts = {}      # (b, i) -> [2, ITILE] denominators (row h)
        rcpbs = {}    # (b, i) -> [2, ITILE] bf16 reciprocals

        def xts(b, tt, c):
            xti = xtp[b * 2 + tt // 2]
            off = c * 1024 + (tt % 2) * ITILE
            return xti[:, off: off + ITILE]

        def qkv_tp(b, tp):
            if b not in qkT:
                q2 = p_qk.tile([128, NSEQ], BF16, tag="q2", name="q2")
                k2 = p_qk.tile([128, NSEQ], BF16, tag="k2", name="k2")
                vT = p_vt.tile([128, NSEQ], BF16, tag="vT", name="vT")
                qkT[b] = (q2, k2)
                vTs[b] = vT
            q2, k2 = qkT[b]
            vT = vTs[b]
            for w in range(2):          # 0 -> q, 1 -> k
                dst = q2 if w == 0 else k2
                pst = [ps_mm.tile([128, ITILE], F32, tag="mm", name="pst")
                       for _ in range(2)]
                for c in range(NKC):
                    for u in range(2):
                        nc.tensor.matmul(
                            pst[u][:],
                            wqk_sb[:, c * 256 + w * 128: c * 256 + (w + 1) * 128],
                            xts(b, 2 * tp + u, c),
                            start=(c == 0), stop=(c == NKC - 1))
                for u in range(2):
                    tsl2 = slice((2 * tp + u) * ITILE, (2 * tp + u + 1) * ITILE)
                    nc.vector.tensor_copy(dst[:, tsl2], pst[u][:])
            pst = [ps_mm.tile([128, ITILE], F32, tag="mm", name="pst")
                   for _ in range(2)]
            for c in range(NKC):
                for u in range(2):
                    nc.tensor.matmul(
                        pst[u][:],
                        wv_sb[:, c * 128:(c + 1) * 128],
                        xts(b, 2 * tp + u, c),
                        start=(c == 0), stop=(c == NKC - 1))
            for u in range(2):
                nc.vector.tensor_copy(
                    vT[:, (2 * tp + u) * ITILE:(2 * tp + u + 1) * ITILE],
                    pst[u][:])

        def vn_block(b, tcns):
            vT = vTs[b]
            vn = vns.setdefault(b, [None] * NJ)
            for tcn in tcns:
                vtr = ps_mm.tile([128, 2 * ITILE], BF16, tag="mm", name="vtr")
                nc.tensor.transpose(vtr[:, 0:128],
                                    vT[:, tcn * 128:(tcn + 1) * 128], ident[:])
                vni = p_v.tile([128, 193], BF16, tag="v", name="vni")
                nc.vector.memset(vni[:, 64:65], 1.0)
                nc.vector.memset(vni[:, 129:130], 1.0)
                nc.vector.memset(vni[:, 130:193], 0.0)
                nc.vector.tensor_copy(vni[:, 0:64], vtr[:, 0:64])
                nc.vector.tensor_copy(vni[:, 65:129], vtr[:, 64:128])
                vn[tcn] = vni

        attn_outu = {}

        def attn_part(b, i, groups):
            q2, k2 = qkT[b]
            vn = vns[b]
            isl = slice(i * ITILE, (i + 1) * ITILE)
            if (b, i) not in attn_outu:
                attn_outu[(b, i)] = [ps_ou.tile([128, ITILE], F32, tag="outu",
                                                name="outu")
                                     for _ in range(2)]
            outu = attn_outu[(b, i)]
            for g in groups:
                # ONE psum tile for BOTH heads' S -> the pair of row-tiled
                # matmuls shares its WAR dependency and stays adjacent in the
                # schedule, so the K=64 tile_position concurrency engages
                s2 = ps_s.tile([128, 2 * JG * ITILE], F32, tag="s",
                               name="sps")
                # 2-head row-tiled S: two concurrent K=64 matmuls on row
                # groups (0,0) and (64,0) -> full PE activity, no zero-pad
                for k in range(JG):
                    j = g * JG + k
                    jsl = slice(j * 128, (j + 1) * 128)
                    nc.tensor.matmul(
                        s2[:, k * ITILE:(k + 1) * ITILE],
                        k2[0:64, jsl], q2[0:64, isl],
                        start=True, stop=True, tile_position=(0, 0))
                    nc.tensor.matmul(
                        s2[:, JG * ITILE + k * ITILE:
                           JG * ITILE + (k + 1) * ITILE],
                        k2[64:128, jsl], q2[64:128, isl],
                        start=True, stop=True, tile_position=(64, 0))
                for h in range(2):
                    ex = p_exp.tile([128, JG * ITILE], BF16, tag=f"ex{h}",
                                    name="ex")
                    nc.scalar.activation(
                        ex[:], s2[:, h * JG * ITILE:(h + 1) * JG * ITILE],
                        EXP, scale=SCALE)
                    for k in range(JG):
                        j = g * JG + k
                        nc.tensor.matmul(
                            outu[h][:],
                            vn[j][:, h * 65: h * 65 + 128],
                            ex[:, k * ITILE:(k + 1) * ITILE],
                            start=(j == 0), stop=(j == NJ - 1))

        def attn_flush(b, i):
            """Flush outu psum -> SBUF; batch both heads' denominators and
            start the reciprocal chain immediately (off the PE queue)."""
            outu = attn_outu.pop((b, i))
            # both heads' denominators packed along the free dim on
            # partition 0 (reciprocal_approx_fast misbehaves off partition 0)
            dt = p_small.tile([1, 2 * ITILE], F32, tag="dt", name="dt")
            for h in range(2):
                ouc = p_ouc.tile([64, ITILE], F32, tag="ouc", name="ouc")
                nc.vector.tensor_copy(ouc[:], outu[h][0:64, :])
                nc.vector.tensor_copy(dt[0:1, h * ITILE:(h + 1) * ITILE],
                                      outu[h][64:65, :])
                oucs[(b, i, h)] = ouc
            rcp = p_small.tile([1, 2 * ITILE], F32, tag="rcp", name="rcp")
            nc.vector.reciprocal_approx_fast(out=rcp[:], in_=dt[:])
            rcpb = p_small.tile([1, 2 * ITILE], BF16, tag="rcpb", name="rcpb")
            nc.vector.tensor_copy(rcpb[:], rcp[:])
            dts[(b, i)] = dt
            rcpbs[(b, i)] = rcpb

        def attn(b, i):
            attn_part(b, i, range(NJ // JG))
            attn_flush(b, i)

        def normB(b, i):
            """Broadcast 1/denom to 64 partitions per head (two K=1 matmuls
            into one psum tile) and scale the numerators into outT."""
            t0 = b * NSEQ
            rcpb = rcpbs.pop((b, i))
            bc2 = ps_mm.tile([128, ITILE], F32, tag="mm", name="bc2")
            nc.tensor.matmul(bc2[0:64, :], ones_row[0:1, 0:64],
                             rcpb[0:1, 0:ITILE], start=True, stop=True)
            nc.tensor.matmul(bc2[64:128, :], ones_row[0:1, 64:128],
                             rcpb[0:1, ITILE:2 * ITILE],
                             start=True, stop=True)
            for h in range(2):
                ouc = oucs.pop((b, i, h))
                nc.vector.tensor_mul(
                    outT[h * 64:(h + 1) * 64,
                         t0 + i * ITILE: t0 + (i + 1) * ITILE],
                    ouc[:], bc2[h * 64:(h + 1) * 64, :])

        def stage_a2a(q):
            hs = CC_HS[q]
            nc.sync.dma_start(
                out=a2a_in[q][:, :].rearrange("(s p) t -> p s t", s=NCORES),
                in_=outT[:, hs: hs + NCORES * CC_TF[q]].rearrange(
                    "p (s t) -> p s t", s=NCORES))
            nc.gpsimd.collective_compute(
                "AllToAll", mybir.AluOpType.bypass,
                replica_groups=[list(range(NCORES))],
                ins=[a2a_in[q][:, :]], outs=[a2a_out[q][:, :]])

        def oproj(q):
            tf = CC_TF[q]
            ots = p_ots.tile([128, NCORES * TFRAG], BF16, tag="ots", name="ots")
            # two gather halves so the contraction can start on chunks 0-3
            # before the whole all-to-all output has landed
            hc = NCORES // 2
            for g2 in range(2):
                nc.sync.dma_start(
                    out=ots[:, g2 * hc * tf:(g2 + 1) * hc * tf].rearrange(
                        "p (s t) -> p s t", s=hc),
                    in_=a2a_out[q][g2 * hc * 128:(g2 + 1) * hc * 128,
                                   :].rearrange("(s p) t -> p s t", s=hc))
            y_ps = [ps_mm.tile([128, ITILE], F32, tag="mm", name="yps")
                    for _ in range(2)]
            for s in range(NKC):
                for n in range(2):
                    nc.tensor.matmul(
                        y_ps[n][0:tf, :],
                        ots[:, s * tf:(s + 1) * tf],
                        wp_sb[:, s * C + n * ITILE: s * C + (n + 1) * ITILE],
                        start=(s == 0), stop=False)
            for n in range(2):
                nc.tensor.matmul(y_ps[n][0:tf, :], ones_row[0:1, 0:tf],
                                 bp_sb[:, n * ITILE:(n + 1) * ITILE],
                                 start=False, stop=True)
            y_sb = p_y.tile([128, C], F32, tag="y", name="ysb")
            for n in range(2):
                nc.vector.tensor_copy(y_sb[0:tf, n * ITILE:(n + 1) * ITILE],
                                      y_ps[n][0:tf, :])
            yr0 = CC_YR[q]
            nc.sync.dma_start(out=y_d[yr0: yr0 + tf, :], in_=y_sb[0:tf, :])

        # ---- software-pipelined emission schedule ----
        G = NJ // JG
        qkv_tp(0, 0)
        vn_block(0, range(NJ // 2))
        attn_part(0, 0, range(G // 2))
        qkv_tp(0, 1)
        vn_block(0, range(NJ // 2, NJ))
        attn_part(0, 0, range(G // 2, G)); attn_flush(0, 0)
        attn(0, 1)
        normB(0, 0); normB(0, 1); stage_a2a(0)
        attn_part(0, 2, range(G // 2))
        qkv_tp(1, 0)
        attn_part(0, 2, range(G // 2, G)); attn_flush(0, 2)
        attn_part(0, 3, range(G // 2)); oproj(0)
        attn_part(0, 3, range(G // 2, G)); attn_flush(0, 3); normB(0, 2)
        qkv_tp(1, 1)
        vn_block(1, range(NJ))
        normB(0, 3); stage_a2a(1)
        attn(1, 0); oproj(1)
        attn(1, 1)
        normB(1, 0); normB(1, 1); stage_a2a(2)
        attn_part(1, 2, range(G // 2))
        attn_part(1, 2, range(G // 2, G)); attn_flush(1, 2)
        attn_part(1, 3, range(G // 2)); oproj(2)
        attn_part(1, 3, range(G // 2, G)); attn_flush(1, 3); normB(1, 2)
        normB(1, 3); stage_a2a(3)
        oproj(3)
        if DEBUG:
            nc.sync.dma_start(out=dbg_d[:, :], in_=outT[:, :])

    nc.compile()
    return nc


_NC = None


def _get_nc():
    global _NC
    if _NC is None:
        _NC = build_program()
    return _NC


def prep_in_maps(x, w_qkv, w_proj, b_proj):
    x_bf = np.ascontiguousarray(np.asarray(x, dtype=np.float32).reshape(T, C)
                                ).astype(NPBF16)
    w_qkv = np.asarray(w_qkv, dtype=np.float32)
    w_proj = np.asarray(w_proj, dtype=np.float32)
    b_proj = np.asarray(b_proj, dtype=np.float32)
    wp_bf = np.ascontiguousarray(w_proj).astype(NPBF16)
    bp_bf = b_proj.reshape(1, C).astype(NPBF16)

    q_w, k_w, v_w = w_qkv[:, 0:C], w_qkv[:, C:2 * C], w_qkv[:, 2 * C:3 * C]
    in_maps = []
    for c in range(NCORES):
        hA, hB = 2 * c, 2 * c + 1
        sA, sB = slice(hA * D, (hA + 1) * D), slice(hB * D, (hB + 1) * D)
        wqk_c = np.concatenate([q_w[:, sA], q_w[:, sB], k_w[:, sA], k_w[:, sB]],
                               axis=1).astype(NPBF16)
        wv_c = np.concatenate([v_w[:, sA], v_w[:, sB]], axis=1).astype(NPBF16)
        in_maps.append({"x": x_bf, "wqk": np.ascontiguousarray(wqk_c),
                        "wv": np.ascontiguousarray(wv_c), "wproj": wp_bf,
                        "bproj": bp_bf})
    return in_maps


CC_TF_H = [TFRAG] * 4
CC_HS_H = [0, HALF, NSEQ, NSEQ + HALF]
CC_YR_H = [0, TFRAG, TSL // B, TSL // B + TFRAG]


def assemble(results):
    y = np.empty((T, C), dtype=np.float32)
    for c in range(NCORES):
        yc = results[c]["y"]
        for q in range(4):
            tf = CC_TF_H[q]
            g0 = CC_HS_H[q] + c * tf
            r0 = CC_YR_H[q]
            y[g0: g0 + tf, :] = yc[r0: r0 + tf, :]
    return y.reshape(B, NSEQ, C)


def run(in_maps, trace=False):
    nc = _get_nc()
    return run_bass_kernel_spmd(nc, in_maps, core_ids=list(range(NCORES)),
                                trace=trace)


def kernel(x, w_qkv, w_proj, b_proj):
    res = run(prep_in_maps(x, w_qkv, w_proj, b_proj))
    return assemble(res.results)
